# revision 11
# baseline (speedup 1.0000x reference)
"""Causal+padding-masked multi-head attention on 8 Trainium2 NeuronCores.

Problem: q[2,16,2048,64], k[2,16,64,2048], v[2,16,2048,64], mask_pad[2,1,1,2048]
-> out[2,16,2048,64] fp32 (softmax((q@k)/8 with pad+causal mask) @ v).

Sharding: batch*head data parallel - 32 (b,h) pairs, 4 per core; cores 0-3
take batch 0, cores 4-7 batch 1 (pad mask replicated per batch shard).

Per-core kernel, per (b,h) pair, all matmuls in fp32r (~1 cyc/row at N>=512,
~1.6e-4 relative error, vs 2e-3 for bf16):
  scoresT[t,s] = sum_d k[d,t]*qT[d,s] + pad_bias[t]   (K=65: row 64 of the
                 k operand holds pad_bias*8, row 64 of qT is ones)
  attT = exp(scoresT/8)  on ScalarE (scale=0.125 folds in 1/sqrt(64), done
         per [128,1024] pair of score tiles to amortize ACT overhead);
         pad-masked t-rows get exp(x-50) ~= 2e-22 (reference has exactly 0;
         contamination ~1e-19 relative).
  causal mask: t-chunks fully above the diagonal are skipped outright;
         diagonal tiles are zeroed exactly with gpsimd affine_select.
  outT[d,s] = sum_t v_ext[t,d]*attT[t,s] accumulated over t-chunks in PSUM;
         v_ext has a ones column so row 64 of outT is the softmax denominator.
  PE-transposes 128-wide slices of outT back to [s,d], VectorE reciprocal +
         per-row scale into a per-(b,h) staging buffer, single DMA out.
  Fully-masked rows (all pad bits 0 up to the diagonal; the reference
  softmaxes a constant row -> uniform 1/2048 -> out = mean(v)): detected at
  runtime via rowsum < 1e-10 and blended with u = mean_t(v) (on-device).
"""
import os
import sys

sys.path.insert(0, "/opt/trn_rl_repo")

import numpy as np

B, H, S, D = 2, 16, 2048, 64
NCORES = 8
BH_PER_CORE = (B * H) // NCORES  # 4
NCHUNK = S // 128   # 16 t-chunks of 128
NBLK = S // 512     # 4 s-blocks of 512
PAD_RAW = -400.0    # pre-scale pad bias; *0.125 -> -50 in the exponent
FIXUP_THRESH = 1e-10


def _register_ntff_shim():
    """The image's antenv lacks axon_hooks; register the NTFF profile hook so
    BASS_TRACE=1 works. Degrades silently if the axon boot pieces are absent."""
    import types
    if "antenv.axon_hooks" in sys.modules:
        return
    try:
        mod = types.ModuleType("antenv.axon_hooks")
        _hook = [None]
        mod.set_axon_ntff_profile_hook = lambda h: _hook.__setitem__(0, h)
        mod.get_axon_ntff_profile_hook = lambda: _hook[0]
        sys.modules["antenv.axon_hooks"] = mod
        import antenv
        antenv.axon_hooks = mod
        if "/root/.axon_site" not in sys.path:
            sys.path.insert(0, "/root/.axon_site")
        from trn_agent_boot.trn_boot import _ntff_profile_via_ctypes
        mod.set_axon_ntff_profile_hook(
            _ntff_profile_via_ctypes("/opt/axon/libaxon_pjrt.so"))
    except Exception:
        pass


def build_program():
    import concourse.bacc as bacc
    import concourse.tile as tile
    import concourse.mybir as mybir
    from concourse import masks

    f32 = mybir.dt.float32
    f32r = mybir.dt.float32r
    AF = mybir.ActivationFunctionType
    ALU = mybir.AluOpType

    nc = bacc.Bacc("TRN2", target_bir_lowering=False, debug=False)

    qt_d = nc.dram_tensor("qt", [BH_PER_CORE, 65, S], f32, kind="ExternalInput")
    kx_d = nc.dram_tensor("kx", [BH_PER_CORE, 65, S], f32, kind="ExternalInput")
    vx_d = nc.dram_tensor("vx", [BH_PER_CORE, 128, NCHUNK, 65], f32, kind="ExternalInput")
    out_d = nc.dram_tensor("out", [BH_PER_CORE, 128, NCHUNK, D], f32, kind="ExternalOutput")

    with tile.TileContext(nc) as tc:
        with (
            tc.tile_pool(name="consts", bufs=1) as consts,
            tc.tile_pool(name="qt", bufs=BH_PER_CORE) as qt_pool,
            tc.tile_pool(name="kx", bufs=BH_PER_CORE) as kx_pool,
            tc.tile_pool(name="vx", bufs=BH_PER_CORE) as vx_pool,
            tc.tile_pool(name="at", bufs=3) as at_pool,
            tc.tile_pool(name="osb", bufs=3) as osb_pool,
            tc.tile_pool(name="ostage", bufs=BH_PER_CORE) as ostage_pool,
            tc.tile_pool(name="small", bufs=6) as small_pool,
            tc.tile_pool(name="ubc", bufs=2) as ubc_pool,
            tc.tile_pool(name="ps_s", bufs=2, space="PSUM") as ps_s,
            tc.tile_pool(name="ps_o", bufs=2, space="PSUM") as ps_o,
            tc.tile_pool(name="ps_u", bufs=1, space="PSUM") as ps_u,
            tc.tile_pool(name="ps_tr", bufs=1, space="PSUM") as ps_tr,
        ):
            ident = consts.tile([128, 128], f32)
            masks.make_identity(nc, ident[:])
            cvec = consts.tile([128, 2], f32r)
            nc.gpsimd.memset(cvec[:].bitcast(f32), 1.0 / S)

            state = {}  # per-l SBUF tiles
            deferred = None  # (l, j, oT_ps) normalize work, emitted one unit late

            def load_bh(l):
                qt_sb = qt_pool.tile([65, S], f32r)
                nc.sync.dma_start(qt_sb[:], qt_d[l].bitcast(f32r))
                kx_sb = kx_pool.tile([65, S], f32r)
                nc.scalar.dma_start(kx_sb[:], kx_d[l].bitcast(f32r))
                vx_sb = vx_pool.tile([128, NCHUNK, 65], f32r)
                nc.sync.dma_start(vx_sb[:], vx_d[l].bitcast(f32r))
                o_stage = ostage_pool.tile([128, NCHUNK, D], f32)
                state[l] = dict(qt=qt_sb, kx=kx_sb, vx=vx_sb, ost=o_stage)

            def normalize(l, j, oT_ps):
                st = state[l]
                oT_sb = osb_pool.tile([65, 512], f32, tag="oT")
                nc.vector.tensor_copy(oT_sb[:], oT_ps[:])
                for q4 in range(4):
                    tr_ps = ps_tr.tile([128, 65], f32, tag="tr")
                    nc.tensor.transpose(
                        tr_ps[:], oT_sb[:, 128 * q4:128 * (q4 + 1)],
                        ident[0:65, 0:65])
                    rcp = small_pool.tile([128, 1], f32, tag="rcp")
                    nc.vector.reciprocal(rcp[:], tr_ps[:, 64:65])
                    dst = st["ost"][:, 4 * j + q4, :]
                    if j == 0 and q4 == 0:
                        # rows whose every key is masked: reference gives
                        # uniform weights -> mean(v). rowsum < 1e-10 can
                        # only happen for such rows (valid rows keep at
                        # least exp(qk/8) >= e^-30 on the diagonal).
                        m_ok = small_pool.tile([128, 1], f32, tag="mok")
                        nc.vector.tensor_scalar(
                            m_ok[:], tr_ps[:, 64:65], FIXUP_THRESH, None,
                            op0=ALU.is_ge)
                        m_bad = small_pool.tile([128, 1], f32, tag="mbad")
                        nc.vector.tensor_scalar(
                            m_bad[:], tr_ps[:, 64:65], FIXUP_THRESH, None,
                            op0=ALU.is_lt)
                        rcpm = small_pool.tile([128, 1], f32, tag="rcpm")
                        nc.vector.tensor_mul(rcpm[:], rcp[:], m_ok[:])
                        o_tmp = osb_pool.tile([128, D], f32, tag="otmp")
                        nc.vector.tensor_scalar_mul(o_tmp[:], tr_ps[:, 0:D], rcpm[:])
                        u_m = osb_pool.tile([128, D], f32, tag="um")
                        nc.vector.tensor_scalar_mul(u_m[:], st["u_bc"][:], m_bad[:])
                        nc.vector.tensor_add(dst, o_tmp[:], u_m[:])
                    else:
                        nc.vector.tensor_scalar_mul(dst, tr_ps[:, 0:D], rcp[:])
                if j == 0:
                    nc.gpsimd.dma_start(out_d[l], st["ost"][:])
                    del state[l]

            for l in range(BH_PER_CORE):
                load_bh(l)
            # all-l j=3 first, then j=2, ... : keeps the PE stream dense so
            # the HAM clock gate stays at 8/8; sparse normalize tails overlap
            # the next unit's compute (deferred by one unit).
            for j in (3, 2, 1, 0):
                for l in range(BH_PER_CORE):
                    st = state[l]
                    qt_sb, kx_sb, vx_sb = st["qt"], st["kx"], st["vx"]
                    if j == 0:
                        # u = mean_t v[t, :]: cvec as stationary operand ->
                        # [2, D], row 0 is u^T in free-dim orientation.
                        u_ps = ps_u.tile([2, D], f32)
                        for c in range(NCHUNK):
                            nc.tensor.matmul(
                                u_ps[:], cvec[:], vx_sb[:, c, 0:D],
                                start=(c == 0), stop=(c == NCHUNK - 1))
                        u1_sb = small_pool.tile([1, D], f32, tag="u1")
                        nc.vector.tensor_copy(u1_sb[:], u_ps[0:1, :])
                        u_bc = ubc_pool.tile([128, D], f32)
                        nc.gpsimd.partition_broadcast(u_bc[:], u1_sb[:])
                        st["u_bc"] = u_bc

                    oT_ps = ps_o.tile([65, 512], f32)
                    nchunks = 4 * j + 4  # t-chunks 0 .. 4j+3 are (partially) unmasked
                    pending = None      # PV runs one pair behind QK/ACT
                    for c0 in range(0, nchunks, 2):
                        sc_ps = ps_s.tile([128, 1024], f32)
                        for ci in range(2):
                            nc.tensor.matmul(
                                sc_ps[:, 512 * ci:512 * (ci + 1)],
                                kx_sb[:, 128 * (c0 + ci):128 * (c0 + ci + 1)],
                                qt_sb[:, 512 * j:512 * (j + 1)],
                                start=True, stop=True)
                        at = at_pool.tile([128, 1024], f32r)
                        nc.scalar.activation(at[:], sc_ps[:], AF.Exp, bias=0.0, scale=0.125)
                        for ci in range(2):
                            c = c0 + ci
                            if c >= 4 * j:
                                # diagonal tile: keep at[t_loc, s_loc] iff
                                # 512j + s_loc >= 128c + t_loc
                                width = 128 * (c - 4 * j)
                                nc.gpsimd.affine_select(
                                    out=at[:, 512 * ci:512 * ci + width + 128],
                                    in_=at[:, 512 * ci:512 * ci + width + 128],
                                    compare_op=ALU.is_ge,
                                    fill=0.0,
                                    base=-width,
                                    pattern=[[1, width + 128]],
                                    channel_multiplier=-1)
                        if pending is not None:
                            pat, pc0 = pending
                            for ci in range(2):
                                c = pc0 + ci
                                nc.tensor.matmul(
                                    oT_ps[:], vx_sb[:, c, :], pat[:, 512 * ci:512 * (ci + 1)],
                                    start=(c == 0), stop=False)
                        pending = (at, c0)
                        if pending is not None and deferred is not None and c0 == 0:
                            normalize(*deferred)
                            deferred = None
                    pat, pc0 = pending
                    for ci in range(2):
                        c = pc0 + ci
                        nc.tensor.matmul(
                            oT_ps[:], vx_sb[:, c, :], pat[:, 512 * ci:512 * (ci + 1)],
                            start=(c == 0), stop=(c == nchunks - 1))
                    deferred = (l, j, oT_ps)
            normalize(*deferred)

    nc.compile()
    return nc


_PROGRAM = None
LAST_RESULTS = None


def kernel(q, k, v, mask_pad):
    global _PROGRAM, LAST_RESULTS
    q = np.ascontiguousarray(np.asarray(q, dtype=np.float32))
    k = np.ascontiguousarray(np.asarray(k, dtype=np.float32))
    v = np.ascontiguousarray(np.asarray(v, dtype=np.float32))
    mask_pad = np.asarray(mask_pad)

    if os.environ.get("BASS_TRACE"):
        _register_ntff_shim()

    pad_bias = np.where(mask_pad[:, 0, 0, :] == 0, np.float32(PAD_RAW), np.float32(0.0))  # [B, S]

    # host-side input staging per core (layouts are partition-major so every
    # DMA packet is one contiguous multi-KB run per partition)
    in_maps = []
    for core in range(NCORES):
        qt = np.empty((BH_PER_CORE, 65, S), np.float32)
        kx = np.empty((BH_PER_CORE, 65, S), np.float32)
        vx = np.empty((BH_PER_CORE, 128, NCHUNK, 65), np.float32)
        for l in range(BH_PER_CORE):
            bh = core * BH_PER_CORE + l
            b, h = bh // H, bh % H
            qt[l, :D] = q[b, h].T
            qt[l, D] = 1.0
            kx[l, :D] = k[b, h]
            kx[l, D] = pad_bias[b]
            vx[l, :, :, :D] = v[b, h].reshape(NCHUNK, 128, D).transpose(1, 0, 2)
            vx[l, :, :, D] = 1.0
        in_maps.append({"qt": qt, "kx": kx, "vx": vx})

    if _PROGRAM is None:
        _PROGRAM = build_program()

    from concourse.bass_utils import run_bass_kernel_spmd
    res = run_bass_kernel_spmd(_PROGRAM, in_maps, core_ids=list(range(NCORES)))
    LAST_RESULTS = res
    if res.exec_time_ns is not None:
        print(f"HW exec time: {res.exec_time_ns} ns")
        if res.profile_json:
            print(f"profile_json: {res.profile_json}")

    out = np.empty((B, H, S, D), np.float32)
    for core in range(NCORES):
        o = res.results[core]["out"]  # [BH_PER_CORE, 128, NCHUNK, D]
        for l in range(BH_PER_CORE):
            bh = core * BH_PER_CORE + l
            b, h = bh // H, bh % H
            out[b, h] = o[l].transpose(1, 0, 2).reshape(S, D)
    return out
